# revision 4
# baseline (speedup 1.0000x reference)
"""GNN mean-aggregation message passing on 8 TRN2 NeuronCores.

out = x + 0.5 * segment_mean(x[src], dst)   for x [N, 128], edge_index [2, E]

Strategy (vertex-cut / destination partitioning, windowed gathers):
  - Nodes (rows of x / out) are sharded across the 8 cores tile-wise (tiles of
    128 dst rows, dealt to cores by descending edge count for balance); each
    core gets the full x in bf16 (replicated) plus only the edges whose
    destination lands in its tiles. No inter-core communication.
  - Edges of each tile are sorted by src and split into 4 buckets, one per
    overlapping 32767-row gather window (bases 0/22411/44822/67233), with cut
    points snapped to multiples of 128 so almost every 128-edge group is full.
    Group counts per (tile position, window) are maxed across cores so the
    SPMD program is identical; short buckets pad with idx=0 / dst=-1.
  - One dma_gather per (octet of 7 tiles, window) pulls the bucket's edge rows
    into SBUF; the 4 windows stripe the 4 SWDGE queues so all 16 DMA rings
    stay busy. Descriptor throughput of the rings is the roofline.
  - Per 128-edge group, a one-hot routing matrix (slot -> local dst row) is
    built with a single 4x-mode DVE tensor_scalar (is_equal against an iota
    row, per-partition dst scalar) and used as the stationary operand of a
    bf16 matmul that segment-sums the gathered rows into the tile's PSUM.
  - The residual x + mean is folded into the same PSUM accumulation via a
    per-tile diagonal stationary diag(2*max(deg,1)) (built on-chip, one
    tensor_scalar each) against the tile's x rows; the epilogue is then a
    single ScalarEngine activation out = inv[p] * psum with inv = 0.5 /
    max(deg, 1), followed by the output DMA.
"""

import sys

sys.path.insert(0, "/opt/trn_rl_repo")

import ml_dtypes
import numpy as np

import concourse.bass as bass
import concourse.bacc as bacc
import concourse.tile as tile
from concourse import mybir

P = 128
N_CORES = 8
WIN = 32767
BASES = (0, 22411, 44822, 67233)
NR = 4  # gather windows / SWDGE queues
OCT = 4  # tiles per PSUM generation (4 banks, double-buffered)
N_QUEUES = 4


def preprocess(x, edge_index, n_cores=N_CORES):
    """Shard/sort/pad edges; build per-core device input maps + meta."""
    x = np.ascontiguousarray(x, dtype=np.float32)
    N, D = x.shape
    src = np.asarray(edge_index[0], dtype=np.int64)
    dst = np.asarray(edge_index[1], dtype=np.int64)

    T_total = (N + P - 1) // P
    TPC = (T_total + n_cores - 1) // n_cores
    T_slots = TPC * n_cores

    tile_of = dst // P
    local = dst - tile_of * P
    E_t = np.bincount(tile_of, minlength=T_slots)

    # deal tiles to (core, position) by descending edge count
    rank = np.argsort(-E_t, kind="stable")
    pos_of_tile = np.empty(T_slots, dtype=np.int64)
    core_of_tile = np.empty(T_slots, dtype=np.int64)
    pos_of_tile[rank] = np.arange(T_slots) // n_cores
    core_of_tile[rank] = np.arange(T_slots) % n_cores

    key = tile_of * (1 << 17) + src
    order = np.argsort(key, kind="stable")
    src_s = src[order]
    local_s = local[order]
    key_s = key[order]

    start_t = np.concatenate([[0], np.cumsum(E_t)])
    bases = np.asarray(BASES, dtype=np.int64)

    cuts = np.zeros((T_slots, NR + 1), dtype=np.int64)
    cuts[:, 0] = start_t[:-1]
    cuts[:, NR] = start_t[1:]
    for r in range(1, NR):
        lo = np.searchsorted(key_s, np.arange(T_slots) * (1 << 17) + bases[r])
        hi = np.searchsorted(
            key_s, np.arange(T_slots) * (1 << 17) + bases[r - 1] + WIN
        )
        ideal = start_t[:-1] + np.round(E_t * r / NR).astype(np.int64)
        snapped = start_t[:-1] + ((ideal - start_t[:-1] + 64) // 128) * 128
        cuts[:, r] = np.clip(snapped, lo, hi)
    for r in range(1, NR + 1):
        cuts[:, r] = np.maximum(cuts[:, r], cuts[:, r - 1])

    counts = np.diff(cuts, axis=1)  # [T_slots, NR]
    Gc = (counts + P - 1) // P
    G = np.zeros((TPC, NR), dtype=np.int64)
    np.maximum.at(G, pos_of_tile, Gc)
    TG = int(G.sum())

    n_oct = (TPC + OCT - 1) // OCT
    goff = np.zeros((TPC, NR), dtype=np.int64)
    gather_list = []  # (octet, r, group offset, S)
    off = 0
    for o in range(n_oct):
        poss = range(o * OCT, min((o + 1) * OCT, TPC))
        for r in range(NR):
            g0 = off
            for t in poss:
                goff[t, r] = off
                off += G[t, r]
            gather_list.append((o, r, int(g0), int(off - g0)))
    assert off == TG

    deg_full = np.bincount(dst, minlength=T_slots * P).astype(np.float32)

    xb = x.astype(ml_dtypes.bfloat16)
    iota = np.ascontiguousarray(
        np.tile(np.arange(D, dtype=np.float32).astype(ml_dtypes.bfloat16), (P, 1))
    )
    pcol = np.ascontiguousarray(np.arange(P, dtype=np.float32).reshape(P, 1))

    per_core = []
    for c in range(n_cores):
        idx_pad = np.zeros(TG * P, dtype=np.int16)
        dst_pad = np.full(TG * P, -1.0, dtype=np.float32)
        degT = np.zeros((P, TPC), dtype=np.float32)
        xr = np.zeros((TPC * P, D), dtype=ml_dtypes.bfloat16)
        for t in np.where(core_of_tile == c)[0]:
            pos = pos_of_tile[t]
            for r in range(NR):
                a, b = cuts[t, r], cuts[t, r + 1]
                n = b - a
                if n == 0:
                    continue
                s0 = goff[pos, r] * P
                idx_pad[s0 : s0 + n] = (src_s[a:b] - bases[r]).astype(np.int16)
                dst_pad[s0 : s0 + n] = local_s[a:b].astype(np.float32)
            r0 = t * P
            r1 = min(N, r0 + P)
            if r0 < N:
                degT[: r1 - r0, pos] = deg_full[r0:r1]
                xr[pos * P : pos * P + (r1 - r0)] = x[r0:r1].astype(
                    ml_dtypes.bfloat16
                )
        wrapped = np.ascontiguousarray(idx_pad.reshape(TG * 8, 16).T)
        idxT = np.ascontiguousarray(np.tile(wrapped, (8, 1)))
        dstT = np.ascontiguousarray(dst_pad.reshape(TG, P).T)
        per_core.append(
            {
                "xb": xb,
                "xr": xr,
                "idxT": idxT,
                "dstT": dstT,
                "degT": degT,
                "iota": iota,
                "pcol": pcol,
            }
        )

    meta = dict(
        G=G,
        TG=TG,
        n_oct=n_oct,
        TPC=TPC,
        gather_list=gather_list,
        goff=goff,
        pos_of_tile=pos_of_tile,
        core_of_tile=core_of_tile,
        T_total=T_total,
        N=N,
        D=D,
    )
    return per_core, meta


def build_core_kernel(meta):
    """Build the per-core Bass program (identical across cores)."""
    G, TG, TPC, n_oct = meta["G"], meta["TG"], meta["TPC"], meta["n_oct"]
    N, D = meta["N"], meta["D"]
    f32 = mybir.dt.float32
    bf16 = mybir.dt.bfloat16
    i16 = mybir.dt.int16

    nc = bacc.Bacc("TRN2", target_bir_lowering=False, num_swdge_queues=N_QUEUES)

    xb_ext = nc.dram_tensor("xb", [N, D], bf16, kind="ExternalInput")
    xr_ext = nc.dram_tensor("xr", [TPC * P, D], bf16, kind="ExternalInput")
    idx_ext = nc.dram_tensor("idxT", [P, TG * 8], i16, kind="ExternalInput")
    dst_ext = nc.dram_tensor("dstT", [P, TG], f32, kind="ExternalInput")
    deg_ext = nc.dram_tensor("degT", [P, TPC], f32, kind="ExternalInput")
    iota_ext = nc.dram_tensor("iota", [P, D], bf16, kind="ExternalInput")
    pcol_ext = nc.dram_tensor("pcol", [P, 1], f32, kind="ExternalInput")
    out_ext = nc.dram_tensor("out", [TPC * P, D], f32, kind="ExternalOutput")

    with tile.TileContext(nc) as tc:
        with (
            tc.tile_pool(name="singles", bufs=1) as singles,
            tc.tile_pool(name="gather", bufs=6) as gpool,
            tc.tile_pool(name="onehot", bufs=24) as ohpool,
            tc.tile_pool(name="acc", bufs=2 * OCT, space="PSUM") as psum,
            tc.tile_pool(name="xres", bufs=6) as xpool,
            tc.tile_pool(name="outp", bufs=6) as opool,
        ):
            idx_sb = singles.tile([P, TG * 8], i16)
            dst_sb = singles.tile([P, TG], f32)
            deg_sb = singles.tile([P, TPC], f32)
            inv_sb = singles.tile([P, TPC], f32)
            wd2_sb = singles.tile([P, TPC], f32)
            iota_sb = singles.tile([P, D], bf16)
            pcol_sb = singles.tile([P, 1], f32)
            wd_sb = singles.tile([P, TPC, D], bf16)

            nc.sync.dma_start(out=idx_sb[:], in_=idx_ext[:])
            nc.sync.dma_start(out=dst_sb[:], in_=dst_ext[:])
            nc.sync.dma_start(out=deg_sb[:], in_=deg_ext[:])
            nc.sync.dma_start(out=iota_sb[:], in_=iota_ext[:])
            nc.sync.dma_start(out=pcol_sb[:], in_=pcol_ext[:])

            # deg' = max(deg, 1); inv = 0.5/deg'; wd2 = 2*deg'
            nc.vector.tensor_scalar(
                out=inv_sb[:], in0=deg_sb[:], scalar1=1.0, scalar2=None,
                op0=mybir.AluOpType.max,
            )
            nc.vector.tensor_scalar(
                out=wd2_sb[:], in0=inv_sb[:], scalar1=2.0, scalar2=None,
                op0=mybir.AluOpType.mult,
            )
            nc.vector.reciprocal(inv_sb[:], inv_sb[:])
            nc.vector.tensor_scalar(
                out=inv_sb[:], in0=inv_sb[:], scalar1=0.5, scalar2=None,
                op0=mybir.AluOpType.mult,
            )
            # prebuild all residual diagonal stationaries:
            # wd[p, t, i] = (i == p) * 2*deg'[p, t]
            for t in range(TPC):
                nc.vector.tensor_scalar(
                    out=wd_sb[:, t, :], in0=iota_sb[:],
                    scalar1=pcol_sb[:], scalar2=wd2_sb[:, t : t + 1],
                    op0=mybir.AluOpType.is_equal, op1=mybir.AluOpType.mult,
                )

            gl = meta["gather_list"]
            gi = 0
            for o in range(n_oct):
                poss = list(range(o * OCT, min((o + 1) * OCT, TPC)))
                pts = {t: psum.tile([P, D], f32, tag="acc", name=f"pt_{t}") for t in poss}
                started = {t: False for t in poss}
                for r in range(NR):
                    o_, r_, g0, S = gl[gi]
                    assert (o_, r_) == (o, r)
                    gi += 1
                    if S == 0:
                        continue
                    gt = gpool.tile([P, S, D], bf16, tag="gather")
                    nc.gpsimd.dma_gather(
                        out_ap=gt[:],
                        in_ap=xb_ext[BASES[r] : min(N, BASES[r] + WIN), :],
                        idxs_ap=idx_sb[:, g0 * 8 : (g0 + S) * 8],
                        num_idxs=S * P,
                        num_idxs_reg=S * P,
                        elem_size=D,
                        single_packet=False,
                        queue_num=r,
                    )
                    s = 0
                    for t in poss:
                        for _ in range(int(G[t, r])):
                            oh = ohpool.tile([P, D], bf16, tag="onehot")
                            nc.vector.tensor_scalar(
                                out=oh[:], in0=iota_sb[:],
                                scalar1=dst_sb[:, g0 + s : g0 + s + 1], scalar2=None,
                                op0=mybir.AluOpType.is_equal,
                            )
                            nc.tensor.matmul(
                                out=pts[t][:],
                                lhsT=oh[:],
                                rhs=gt[:, s, :],
                                start=not started[t],
                                stop=False,
                            )
                            started[t] = True
                            s += 1

                for t in poss:
                    xt = xpool.tile([P, D], bf16, tag="xres")
                    nc.sync.dma_start(
                        out=xt[:], in_=xr_ext[t * P : (t + 1) * P, :]
                    )
                    nc.tensor.matmul(
                        out=pts[t][:],
                        lhsT=wd_sb[:, t, :],
                        rhs=xt[:],
                        start=not started[t],
                        stop=True,
                    )
                    ot = opool.tile([P, D], f32, tag="outp")
                    nc.scalar.activation(
                        out=ot[:], in_=pts[t][:],
                        func=mybir.ActivationFunctionType.Copy,
                        scale=inv_sb[:, t : t + 1],
                    )
                    nc.sync.dma_start(
                        out=out_ext[t * P : (t + 1) * P, :], in_=ot[:]
                    )

    nc.compile()
    return nc


def kernel(x, edge_index):
    from concourse.bass_utils import run_bass_kernel_spmd

    x = np.ascontiguousarray(np.asarray(x), dtype=np.float32)
    per_core, meta = preprocess(x, edge_index, N_CORES)
    nc = build_core_kernel(meta)
    res = run_bass_kernel_spmd(nc, per_core, core_ids=list(range(N_CORES)))

    N, D, TPC = meta["N"], meta["D"], meta["TPC"]
    out = np.empty((N, D), dtype=np.float32)
    pos_of_tile = meta["pos_of_tile"]
    core_of_tile = meta["core_of_tile"]
    for t in range(meta["T_total"]):
        c = core_of_tile[t]
        pos = pos_of_tile[t]
        r0 = t * P
        r1 = min(N, r0 + P)
        out[r0:r1] = res.results[c]["out"][pos * P : pos * P + (r1 - r0)]
    return out


# revision 7
# speedup vs baseline: 1.9383x; 1.9383x over previous
"""GNN mean-aggregation message passing on 8 TRN2 NeuronCores.

out = x + 0.5 * segment_mean(x[src], dst)   for x [N, 128], edge_index [2, E]

Strategy (vertex-cut / destination partitioning, windowed gathers):
  - Nodes (rows of x / out) are sharded across the 8 cores tile-wise (tiles of
    128 dst rows, dealt to cores by descending edge count for balance); each
    core gets the full x in bf16 (replicated) plus only the edges whose
    destination lands in its tiles. No inter-core communication.
  - Edges of each tile are sorted by src and split into 4 buckets, one per
    overlapping 32767-row gather window (bases 0/22411/44822/67233), with cut
    points snapped to multiples of 128 so almost every 128-edge group is full.
    Group counts per (tile position, window) are maxed across cores so the
    SPMD program is identical; short buckets pad with idx=0 / dst=-1.
  - One dma_gather per (octet of 7 tiles, window) pulls the bucket's edge rows
    into SBUF; the 4 windows stripe the 4 SWDGE queues so all 16 DMA rings
    stay busy. Descriptor throughput of the rings is the roofline.
  - Per 128-edge group, a one-hot routing matrix (slot -> local dst row) is
    built with a single 4x-mode DVE tensor_scalar (is_equal against an iota
    row, per-partition dst scalar) and used as the stationary operand of a
    bf16 matmul that segment-sums the gathered rows into the tile's PSUM.
  - The residual x + mean is folded into the same PSUM accumulation via a
    per-tile diagonal stationary diag(2*max(deg,1)) (built on-chip, one
    tensor_scalar each) against the tile's x rows; the epilogue is then a
    single ScalarEngine activation out = inv[p] * psum with inv = 0.5 /
    max(deg, 1), followed by the output DMA.
"""

import sys

sys.path.insert(0, "/opt/trn_rl_repo")

import ml_dtypes
import numpy as np

import concourse.bass as bass
import concourse.bacc as bacc
import concourse.tile as tile
from concourse import mybir

P = 128
N_CORES = 8
WIN = 32767
BASES = (0, 22411, 44822, 67233)
NR = 4  # gather windows / SWDGE queues
OCT = 4  # tiles per PSUM generation (4 banks, double-buffered)
N_QUEUES = 4


def preprocess(x, edge_index, n_cores=N_CORES):
    """Shard/sort/pad edges; build per-core device input maps + meta."""
    x = np.ascontiguousarray(x, dtype=np.float32)
    N, D = x.shape
    src = np.asarray(edge_index[0], dtype=np.int64)
    dst = np.asarray(edge_index[1], dtype=np.int64)

    T_total = (N + P - 1) // P
    TPC = (T_total + n_cores - 1) // n_cores
    T_slots = TPC * n_cores

    tile_of = dst // P
    local = dst - tile_of * P
    E_t = np.bincount(tile_of, minlength=T_slots)

    # deal tiles to (core, position) by descending edge count
    rank = np.argsort(-E_t, kind="stable")
    pos_of_tile = np.empty(T_slots, dtype=np.int64)
    core_of_tile = np.empty(T_slots, dtype=np.int64)
    pos_of_tile[rank] = np.arange(T_slots) // n_cores
    core_of_tile[rank] = np.arange(T_slots) % n_cores

    key = tile_of * (1 << 17) + src
    order = np.argsort(key, kind="stable")
    src_s = src[order]
    local_s = local[order]
    key_s = key[order]

    start_t = np.concatenate([[0], np.cumsum(E_t)])
    bases = np.asarray(BASES, dtype=np.int64)

    cuts = np.zeros((T_slots, NR + 1), dtype=np.int64)
    cuts[:, 0] = start_t[:-1]
    cuts[:, NR] = start_t[1:]
    for r in range(1, NR):
        lo = np.searchsorted(key_s, np.arange(T_slots) * (1 << 17) + bases[r])
        hi = np.searchsorted(
            key_s, np.arange(T_slots) * (1 << 17) + bases[r - 1] + WIN
        )
        ideal = start_t[:-1] + np.round(E_t * r / NR).astype(np.int64)
        snapped = start_t[:-1] + ((ideal - start_t[:-1] + 64) // 128) * 128
        cuts[:, r] = np.clip(snapped, lo, hi)
    for r in range(1, NR + 1):
        cuts[:, r] = np.maximum(cuts[:, r], cuts[:, r - 1])

    counts = np.diff(cuts, axis=1)  # [T_slots, NR]
    Gc = (counts + P - 1) // P
    G = np.zeros((TPC, NR), dtype=np.int64)
    np.maximum.at(G, pos_of_tile, Gc)
    TG = int(G.sum())

    n_oct = (TPC + OCT - 1) // OCT
    goff = np.zeros((TPC, NR), dtype=np.int64)
    gather_list = []  # (octet, r, group offset, S) per (octet, window)
    off = 0
    for o in range(n_oct):
        poss = range(o * OCT, min((o + 1) * OCT, TPC))
        for r in range(NR):
            g0 = off
            for t in poss:
                goff[t, r] = off
                off += G[t, r]
            gather_list.append((o, r, int(g0), int(off - g0)))
    assert off == TG
    # half-gather granularity: split each (o, r) phase into two runs of whole
    # tiles so matmuls wait on half the drain latency
    half_runs = []  # (o, r, g0, S, half_index)
    for o in range(n_oct):
        poss = list(range(o * OCT, min((o + 1) * OCT, TPC)))
        mid = (len(poss) + 1) // 2
        for r in range(NR):
            for h, tl in enumerate((poss[:mid], poss[mid:])):
                if not tl:
                    continue
                g0 = int(goff[tl[0], r])
                S = int(sum(G[t, r] for t in tl))
                half_runs.append((o, r, g0, S, h, tl))
    S_max = max(s for (_, _, _, s, _, _) in half_runs)

    deg_full = np.bincount(dst, minlength=T_slots * P).astype(np.float32)

    xb = x.astype(ml_dtypes.bfloat16)
    iota = np.ascontiguousarray(
        np.tile(np.arange(D, dtype=np.float32).astype(ml_dtypes.bfloat16), (P, 1))
    )
    pcol = np.ascontiguousarray(np.arange(P, dtype=np.float32).reshape(P, 1))
    iota_tiled = np.ascontiguousarray(
        np.tile(
            np.arange(D, dtype=np.float32).astype(ml_dtypes.bfloat16),
            (P, S_max),
        )
    )  # [P, S_max*D]

    per_core = []
    for c in range(n_cores):
        idx_pad = np.zeros(TG * P, dtype=np.int16)
        dst_pad = np.full(TG * P, -1.0, dtype=np.float32)
        degT = np.zeros((P, TPC), dtype=np.float32)
        xr = np.zeros((TPC * P, D), dtype=ml_dtypes.bfloat16)
        for t in np.where(core_of_tile == c)[0]:
            pos = pos_of_tile[t]
            for r in range(NR):
                a, b = cuts[t, r], cuts[t, r + 1]
                n = b - a
                if n == 0:
                    continue
                s0 = goff[pos, r] * P
                idx_pad[s0 : s0 + n] = (src_s[a:b] - bases[r]).astype(np.int16)
                dst_pad[s0 : s0 + n] = local_s[a:b].astype(np.float32)
            r0 = t * P
            r1 = min(N, r0 + P)
            if r0 < N:
                degT[: r1 - r0, pos] = deg_full[r0:r1]
                xr[pos * P : pos * P + (r1 - r0)] = x[r0:r1].astype(
                    ml_dtypes.bfloat16
                )
        wrapped = np.ascontiguousarray(idx_pad.reshape(TG * 8, 16).T)
        idxT = np.ascontiguousarray(np.tile(wrapped, (8, 1)))
        dstT = np.ascontiguousarray(dst_pad.reshape(TG, P).T.astype(ml_dtypes.bfloat16))
        per_core.append(
            {
                "xb": xb,
                "xr": xr,
                "idxT": idxT,
                "dstT": dstT,
                "degT": degT,
                "iota": iota,
                "iota_tiled": iota_tiled,
                "pcol": pcol,
            }
        )

    meta = dict(
        G=G,
        TG=TG,
        half_runs=half_runs,
        S_max=S_max,
        n_oct=n_oct,
        TPC=TPC,
        gather_list=gather_list,
        goff=goff,
        pos_of_tile=pos_of_tile,
        core_of_tile=core_of_tile,
        T_total=T_total,
        N=N,
        D=D,
    )
    return per_core, meta


def build_core_kernel(meta):
    """Build the per-core Bass program (identical across cores)."""
    G, TG, TPC, n_oct = meta["G"], meta["TG"], meta["TPC"], meta["n_oct"]
    N, D = meta["N"], meta["D"]
    S_max = meta["S_max"]
    f32 = mybir.dt.float32
    bf16 = mybir.dt.bfloat16
    i16 = mybir.dt.int16

    nc = bacc.Bacc("TRN2", target_bir_lowering=False, num_swdge_queues=N_QUEUES)

    xb_ext = nc.dram_tensor("xb", [N, D], bf16, kind="ExternalInput")
    xr_ext = nc.dram_tensor("xr", [TPC * P, D], bf16, kind="ExternalInput")
    idx_ext = nc.dram_tensor("idxT", [P, TG * 8], i16, kind="ExternalInput")
    dst_ext = nc.dram_tensor("dstT", [P, TG], bf16, kind="ExternalInput")
    deg_ext = nc.dram_tensor("degT", [P, TPC], f32, kind="ExternalInput")
    iota_ext = nc.dram_tensor("iota", [P, D], bf16, kind="ExternalInput")
    iotat_ext = nc.dram_tensor("iota_tiled", [P, S_max * D], bf16, kind="ExternalInput")
    pcol_ext = nc.dram_tensor("pcol", [P, 1], f32, kind="ExternalInput")
    out_ext = nc.dram_tensor("out", [TPC * P, D], f32, kind="ExternalOutput")

    with tile.TileContext(nc) as tc:
        with (
            tc.tile_pool(name="singles", bufs=1) as singles,
            tc.tile_pool(name="gather", bufs=12) as gpool,
            tc.tile_pool(name="onehot", bufs=10) as ohpool,
            tc.tile_pool(name="acc", bufs=2 * OCT, space="PSUM") as psum,
            tc.tile_pool(name="xres", bufs=6) as xpool,
            tc.tile_pool(name="outp", bufs=6) as opool,
        ):
            idx_sb = singles.tile([P, TG * 8], i16)
            dst_sb = singles.tile([P, TG], bf16)
            deg_sb = singles.tile([P, TPC], f32)
            inv_sb = singles.tile([P, TPC], f32)
            wd2_sb = singles.tile([P, TPC], f32)
            iota_sb = singles.tile([P, D], bf16)
            iotat_sb = singles.tile([P, S_max * D], bf16)
            pcol_sb = singles.tile([P, 1], f32)
            wd_sb = singles.tile([P, TPC, D], bf16)

            nc.sync.dma_start(out=idx_sb[:], in_=idx_ext[:])
            nc.sync.dma_start(out=dst_sb[:], in_=dst_ext[:])
            nc.sync.dma_start(out=deg_sb[:], in_=deg_ext[:])
            nc.sync.dma_start(out=iota_sb[:], in_=iota_ext[:])
            nc.sync.dma_start(out=iotat_sb[:], in_=iotat_ext[:])
            nc.sync.dma_start(out=pcol_sb[:], in_=pcol_ext[:])

            # deg' = max(deg, 1); inv = 0.5/deg'; wd2 = 2*deg'
            nc.vector.tensor_scalar(
                out=inv_sb[:], in0=deg_sb[:], scalar1=1.0, scalar2=None,
                op0=mybir.AluOpType.max,
            )
            nc.vector.tensor_scalar(
                out=wd2_sb[:], in0=inv_sb[:], scalar1=2.0, scalar2=None,
                op0=mybir.AluOpType.mult,
            )
            nc.vector.reciprocal(inv_sb[:], inv_sb[:])
            nc.vector.tensor_scalar(
                out=inv_sb[:], in0=inv_sb[:], scalar1=0.5, scalar2=None,
                op0=mybir.AluOpType.mult,
            )
            # prebuild all residual diagonal stationaries:
            # wd[p, t, i] = (i == p) * 2*deg'[p, t]
            for t in range(TPC):
                nc.vector.tensor_scalar(
                    out=wd_sb[:, t, :], in0=iota_sb[:],
                    scalar1=pcol_sb[:], scalar2=wd2_sb[:, t : t + 1],
                    op0=mybir.AluOpType.is_equal, op1=mybir.AluOpType.mult,
                )

            runs = meta["half_runs"]
            ri = 0
            for o in range(n_oct):
                poss = list(range(o * OCT, min((o + 1) * OCT, TPC)))
                pts = {t: psum.tile([P, D], f32, tag="acc", name=f"pt_{t}") for t in poss}
                started = {t: False for t in poss}
                # all runs of this octet, in (r, half) order
                while ri < len(runs) and runs[ri][0] == o:
                    _, r, g0, S, h, tl = runs[ri]
                    ri += 1
                    if S == 0:
                        continue
                    gt = gpool.tile([P, S, D], bf16, tag="gather")
                    nc.gpsimd.dma_gather(
                        out_ap=gt[:],
                        in_ap=xb_ext[BASES[r] : min(N, BASES[r] + WIN), :],
                        idxs_ap=idx_sb[:, g0 * 8 : (g0 + S) * 8],
                        num_idxs=S * P,
                        num_idxs_reg=S * P,
                        elem_size=D,
                        single_packet=False,
                        queue_num=r,
                    )
                    # batched one-hot for the whole run:
                    # oh[p, s, d] = (iota_tiled[p, s*D+d] == dst[p, g0+s])
                    oh = ohpool.tile([P, S, D], bf16, tag="onehot")
                    dst_ap = dst_sb[:, g0 : g0 + S]
                    dst_b = bass.AP(
                        tensor=dst_ap.tensor,
                        offset=dst_ap.offset,
                        ap=[dst_ap.ap[0], dst_ap.ap[1], [0, D]],
                    )
                    iot_ap = iotat_sb[:]
                    iot_b = bass.AP(
                        tensor=iot_ap.tensor,
                        offset=iot_ap.offset,
                        ap=[iot_ap.ap[0], [D, S], [1, D]],
                    )
                    nc.vector.tensor_tensor(
                        out=oh[:], in0=iot_b, in1=dst_b,
                        op=mybir.AluOpType.is_equal,
                    )
                    s = 0
                    for t in tl:
                        for _ in range(int(G[t, r])):
                            nc.tensor.matmul(
                                out=pts[t][:],
                                lhsT=oh[:, s, :],
                                rhs=gt[:, s, :],
                                start=not started[t],
                                stop=False,
                            )
                            started[t] = True
                            s += 1

                for t in poss:
                    xt = xpool.tile([P, D], bf16, tag="xres")
                    nc.sync.dma_start(
                        out=xt[:], in_=xr_ext[t * P : (t + 1) * P, :]
                    )
                    nc.tensor.matmul(
                        out=pts[t][:],
                        lhsT=wd_sb[:, t, :],
                        rhs=xt[:],
                        start=not started[t],
                        stop=True,
                    )
                    ot = opool.tile([P, D], f32, tag="outp")
                    nc.scalar.activation(
                        out=ot[:], in_=pts[t][:],
                        func=mybir.ActivationFunctionType.Copy,
                        scale=inv_sb[:, t : t + 1],
                    )
                    nc.sync.dma_start(
                        out=out_ext[t * P : (t + 1) * P, :], in_=ot[:]
                    )

    nc.compile()
    return nc


def kernel(x, edge_index):
    from concourse.bass_utils import run_bass_kernel_spmd

    x = np.ascontiguousarray(np.asarray(x), dtype=np.float32)
    per_core, meta = preprocess(x, edge_index, N_CORES)
    nc = build_core_kernel(meta)
    res = run_bass_kernel_spmd(nc, per_core, core_ids=list(range(N_CORES)))

    N, D, TPC = meta["N"], meta["D"], meta["TPC"]
    out = np.empty((N, D), dtype=np.float32)
    pos_of_tile = meta["pos_of_tile"]
    core_of_tile = meta["core_of_tile"]
    for t in range(meta["T_total"]):
        c = core_of_tile[t]
        pos = pos_of_tile[t]
        r0 = t * P
        r1 = min(N, r0 + P)
        out[r0:r1] = res.results[c]["out"][pos * P : pos * P + (r1 - r0)]
    return out


# revision 8
# speedup vs baseline: 1.9929x; 1.0282x over previous
"""GNN mean-aggregation message passing on 8 TRN2 NeuronCores.

out = x + 0.5 * segment_mean(x[src], dst)   for x [N, 128], edge_index [2, E]

Strategy (vertex-cut / destination partitioning, windowed gathers):
  - Nodes (rows of x / out) are sharded across the 8 cores tile-wise (tiles of
    128 dst rows, dealt to cores by descending edge count for balance); each
    core gets the full x in bf16 (replicated) plus only the edges whose
    destination lands in its tiles. No inter-core communication.
  - Edges of each tile are sorted by src and split into 4 buckets, one per
    overlapping 32767-row gather window (bases 0/22411/44822/67233), with cut
    points snapped to multiples of 128 so almost every 128-edge group is full.
    Group counts per (tile position, window) are maxed across cores so the
    SPMD program is identical; short buckets pad with idx=0 / dst=-1.
  - One dma_gather per (octet of 7 tiles, window) pulls the bucket's edge rows
    into SBUF; the 4 windows stripe the 4 SWDGE queues so all 16 DMA rings
    stay busy. Descriptor throughput of the rings is the roofline.
  - Per 128-edge group, a one-hot routing matrix (slot -> local dst row) is
    built with a single 4x-mode DVE tensor_scalar (is_equal against an iota
    row, per-partition dst scalar) and used as the stationary operand of a
    bf16 matmul that segment-sums the gathered rows into the tile's PSUM.
  - The residual x + mean is folded into the same PSUM accumulation via a
    per-tile diagonal stationary diag(2*max(deg,1)) (built on-chip, one
    tensor_scalar each) against the tile's x rows; the epilogue is then a
    single ScalarEngine activation out = inv[p] * psum with inv = 0.5 /
    max(deg, 1), followed by the output DMA.
"""

import sys

sys.path.insert(0, "/opt/trn_rl_repo")

import ml_dtypes
import numpy as np

import concourse.bass as bass
import concourse.bacc as bacc
import concourse.tile as tile
from concourse import mybir

P = 128
N_CORES = 8
WIN = 32767
BASES = (0, 22411, 44822, 67233)
NR = 4  # gather windows / SWDGE queues
OCT = 4  # tiles per PSUM generation (4 banks, double-buffered)
N_QUEUES = 4


def preprocess(x, edge_index, n_cores=N_CORES):
    """Shard/sort/pad edges; build per-core device input maps + meta."""
    x = np.ascontiguousarray(x, dtype=np.float32)
    N, D = x.shape
    src = np.asarray(edge_index[0], dtype=np.int64)
    dst = np.asarray(edge_index[1], dtype=np.int64)

    T_total = (N + P - 1) // P
    TPC = (T_total + n_cores - 1) // n_cores
    T_slots = TPC * n_cores

    tile_of = dst // P
    local = dst - tile_of * P
    E_t = np.bincount(tile_of, minlength=T_slots)

    # deal tiles to (core, position) by descending edge count
    rank = np.argsort(-E_t, kind="stable")
    pos_of_tile = np.empty(T_slots, dtype=np.int64)
    core_of_tile = np.empty(T_slots, dtype=np.int64)
    pos_of_tile[rank] = np.arange(T_slots) // n_cores
    core_of_tile[rank] = np.arange(T_slots) % n_cores

    key = tile_of * (1 << 17) + src
    order = np.argsort(key, kind="stable")
    src_s = src[order]
    local_s = local[order]
    key_s = key[order]

    start_t = np.concatenate([[0], np.cumsum(E_t)])
    bases = np.asarray(BASES, dtype=np.int64)

    cuts = np.zeros((T_slots, NR + 1), dtype=np.int64)
    cuts[:, 0] = start_t[:-1]
    cuts[:, NR] = start_t[1:]
    for r in range(1, NR):
        lo = np.searchsorted(key_s, np.arange(T_slots) * (1 << 17) + bases[r])
        hi = np.searchsorted(
            key_s, np.arange(T_slots) * (1 << 17) + bases[r - 1] + WIN
        )
        ideal = start_t[:-1] + np.round(E_t * r / NR).astype(np.int64)
        snapped = start_t[:-1] + ((ideal - start_t[:-1] + 64) // 128) * 128
        cuts[:, r] = np.clip(snapped, lo, hi)
    for r in range(1, NR + 1):
        cuts[:, r] = np.maximum(cuts[:, r], cuts[:, r - 1])

    counts = np.diff(cuts, axis=1)  # [T_slots, NR]
    Gc = (counts + P - 1) // P
    G = np.zeros((TPC, NR), dtype=np.int64)
    np.maximum.at(G, pos_of_tile, Gc)
    TG = int(G.sum())

    n_oct = (TPC + OCT - 1) // OCT
    goff = np.zeros((TPC, NR), dtype=np.int64)
    gather_list = []  # (octet, r, group offset, S) per (octet, window)
    off = 0
    for o in range(n_oct):
        poss = range(o * OCT, min((o + 1) * OCT, TPC))
        for r in range(NR):
            g0 = off
            for t in poss:
                goff[t, r] = off
                off += G[t, r]
            gather_list.append((o, r, int(g0), int(off - g0)))
    assert off == TG
    # half-gather granularity: split each (o, r) phase into two runs of whole
    # tiles so matmuls wait on half the drain latency
    half_runs = []  # (o, r, g0, S, half_index)
    for o in range(n_oct):
        poss = list(range(o * OCT, min((o + 1) * OCT, TPC)))
        mid = (len(poss) + 1) // 2
        for r in range(NR):
            for h, tl in enumerate((poss[:mid], poss[mid:])):
                if not tl:
                    continue
                g0 = int(goff[tl[0], r])
                S = int(sum(G[t, r] for t in tl))
                half_runs.append((o, r, g0, S, h, tl))
    S_max = max(s for (_, _, _, s, _, _) in half_runs)

    deg_full = np.bincount(dst, minlength=T_slots * P).astype(np.float32)

    xb = x.astype(ml_dtypes.bfloat16)
    iota = np.ascontiguousarray(
        np.tile(np.arange(D, dtype=np.float32).astype(ml_dtypes.bfloat16), (P, 1))
    )
    pcol = np.ascontiguousarray(np.arange(P, dtype=np.float32).reshape(P, 1))
    iota_cs = np.ascontiguousarray(
        np.tile(
            np.repeat(np.arange(D, dtype=np.float32), S_max).astype(
                ml_dtypes.bfloat16
            ),
            (P, 1),
        )
    )  # [P, D*S_max]: (p, d*S_max + s) = d

    per_core = []
    for c in range(n_cores):
        idx_pad = np.zeros(TG * P, dtype=np.int16)
        dst_pad = np.full(TG * P, -1.0, dtype=np.float32)
        degT = np.zeros((P, TPC), dtype=np.float32)
        xr = np.zeros((TPC * P, D), dtype=ml_dtypes.bfloat16)
        for t in np.where(core_of_tile == c)[0]:
            pos = pos_of_tile[t]
            for r in range(NR):
                a, b = cuts[t, r], cuts[t, r + 1]
                n = b - a
                if n == 0:
                    continue
                s0 = goff[pos, r] * P
                idx_pad[s0 : s0 + n] = (src_s[a:b] - bases[r]).astype(np.int16)
                dst_pad[s0 : s0 + n] = local_s[a:b].astype(np.float32)
            r0 = t * P
            r1 = min(N, r0 + P)
            if r0 < N:
                degT[: r1 - r0, pos] = deg_full[r0:r1]
                xr[pos * P : pos * P + (r1 - r0)] = x[r0:r1].astype(
                    ml_dtypes.bfloat16
                )
        wrapped = np.ascontiguousarray(idx_pad.reshape(TG * 8, 16).T)
        idxT = np.ascontiguousarray(np.tile(wrapped, (8, 1)))
        dstT = np.ascontiguousarray(dst_pad.reshape(TG, P).T.astype(ml_dtypes.bfloat16))
        per_core.append(
            {
                "xb": xb,
                "xr": xr,
                "idxT": idxT,
                "dstT": dstT,
                "degT": degT,
                "iota": iota,
                "iota_cs": iota_cs,
                "pcol": pcol,
            }
        )

    meta = dict(
        G=G,
        TG=TG,
        half_runs=half_runs,
        S_max=S_max,
        n_oct=n_oct,
        TPC=TPC,
        gather_list=gather_list,
        goff=goff,
        pos_of_tile=pos_of_tile,
        core_of_tile=core_of_tile,
        T_total=T_total,
        N=N,
        D=D,
    )
    return per_core, meta


def build_core_kernel(meta):
    """Build the per-core Bass program (identical across cores)."""
    G, TG, TPC, n_oct = meta["G"], meta["TG"], meta["TPC"], meta["n_oct"]
    N, D = meta["N"], meta["D"]
    S_max = meta["S_max"]
    f32 = mybir.dt.float32
    bf16 = mybir.dt.bfloat16
    i16 = mybir.dt.int16

    nc = bacc.Bacc("TRN2", target_bir_lowering=False, num_swdge_queues=N_QUEUES)

    xb_ext = nc.dram_tensor("xb", [N, D], bf16, kind="ExternalInput")
    xr_ext = nc.dram_tensor("xr", [TPC * P, D], bf16, kind="ExternalInput")
    idx_ext = nc.dram_tensor("idxT", [P, TG * 8], i16, kind="ExternalInput")
    dst_ext = nc.dram_tensor("dstT", [P, TG], bf16, kind="ExternalInput")
    deg_ext = nc.dram_tensor("degT", [P, TPC], f32, kind="ExternalInput")
    iota_ext = nc.dram_tensor("iota", [P, D], bf16, kind="ExternalInput")
    iotat_ext = nc.dram_tensor("iota_cs", [P, D * S_max], bf16, kind="ExternalInput")
    pcol_ext = nc.dram_tensor("pcol", [P, 1], f32, kind="ExternalInput")
    out_ext = nc.dram_tensor("out", [TPC * P, D], f32, kind="ExternalOutput")

    with tile.TileContext(nc) as tc:
        with (
            tc.tile_pool(name="singles", bufs=1) as singles,
            tc.tile_pool(name="gather", bufs=16) as gpool,
            tc.tile_pool(name="onehot", bufs=10) as ohpool,
            tc.tile_pool(name="acc", bufs=2 * OCT, space="PSUM") as psum,
            tc.tile_pool(name="xres", bufs=6) as xpool,
            tc.tile_pool(name="outp", bufs=6) as opool,
        ):
            idx_sb = singles.tile([P, TG * 8], i16)
            dst_sb = singles.tile([P, TG], bf16)
            deg_sb = singles.tile([P, TPC], f32)
            inv_sb = singles.tile([P, TPC], f32)
            wd2_sb = singles.tile([P, TPC], f32)
            iota_sb = singles.tile([P, D], bf16)
            iotat_sb = singles.tile([P, D * S_max], bf16)
            pcol_sb = singles.tile([P, 1], f32)
            wd_sb = singles.tile([P, TPC, D], bf16)

            nc.sync.dma_start(out=deg_sb[:], in_=deg_ext[:])
            nc.sync.dma_start(out=pcol_sb[:], in_=pcol_ext[:])
            nc.sync.dma_start(out=iota_sb[:], in_=iota_ext[:])
            nc.sync.dma_start(out=iotat_sb[:], in_=iotat_ext[:])
            nc.sync.dma_start(out=dst_sb[:], in_=dst_ext[:])
            nc.sync.dma_start(out=idx_sb[:], in_=idx_ext[:])

            # deg' = max(deg, 1); inv = 0.5/deg'; wd2 = 2*deg'
            nc.vector.tensor_scalar(
                out=inv_sb[:], in0=deg_sb[:], scalar1=1.0, scalar2=None,
                op0=mybir.AluOpType.max,
            )
            nc.vector.tensor_scalar(
                out=wd2_sb[:], in0=inv_sb[:], scalar1=2.0, scalar2=None,
                op0=mybir.AluOpType.mult,
            )
            nc.vector.reciprocal(inv_sb[:], inv_sb[:])
            nc.vector.tensor_scalar(
                out=inv_sb[:], in0=inv_sb[:], scalar1=0.5, scalar2=None,
                op0=mybir.AluOpType.mult,
            )
            # prebuild all residual diagonal stationaries:
            # wd[p, t, i] = (i == p) * 2*deg'[p, t]
            for t in range(TPC):
                nc.vector.tensor_scalar(
                    out=wd_sb[:, t, :], in0=iota_sb[:],
                    scalar1=pcol_sb[:], scalar2=wd2_sb[:, t : t + 1],
                    op0=mybir.AluOpType.is_equal, op1=mybir.AluOpType.mult,
                )

            runs = meta["half_runs"]
            ri = 0
            for o in range(n_oct):
                poss = list(range(o * OCT, min((o + 1) * OCT, TPC)))
                pts = {t: psum.tile([P, D], f32, tag="acc", name=f"pt_{t}") for t in poss}
                started = {t: False for t in poss}
                # all runs of this octet, in (r, half) order
                while ri < len(runs) and runs[ri][0] == o:
                    _, r, g0, S, h, tl = runs[ri]
                    ri += 1
                    if S == 0:
                        continue
                    gt = gpool.tile([P, S, D], bf16, tag="gather")
                    nc.gpsimd.dma_gather(
                        out_ap=gt[:],
                        in_ap=xb_ext[BASES[r] : min(N, BASES[r] + WIN), :],
                        idxs_ap=idx_sb[:, g0 * 8 : (g0 + S) * 8],
                        num_idxs=S * P,
                        num_idxs_reg=S * P,
                        elem_size=D,
                        single_packet=False,
                        queue_num=r,
                    )
                    # batched one-hot for the whole run:
                    # oh[p, s, d] = (iota_tiled[p, s*D+d] == dst[p, g0+s])
                    oh = ohpool.tile([P, D, S], bf16, tag="onehot")
                    dst_ap = dst_sb[:, g0 : g0 + S]
                    dst_b = bass.AP(
                        tensor=dst_ap.tensor,
                        offset=dst_ap.offset,
                        ap=[dst_ap.ap[0], [0, D], dst_ap.ap[1]],
                    )
                    iot_ap = iotat_sb[:]
                    iot_b = bass.AP(
                        tensor=iot_ap.tensor,
                        offset=iot_ap.offset,
                        ap=[iot_ap.ap[0], [S_max, D], [1, S]],
                    )
                    nc.vector.tensor_tensor(
                        out=oh[:], in0=iot_b, in1=dst_b,
                        op=mybir.AluOpType.is_equal,
                    )
                    s = 0
                    for t in tl:
                        for _ in range(int(G[t, r])):
                            nc.tensor.matmul(
                                out=pts[t][:],
                                lhsT=oh[:, :, s],
                                rhs=gt[:, s, :],
                                start=not started[t],
                                stop=False,
                            )
                            started[t] = True
                            s += 1

                for t in poss:
                    xt = xpool.tile([P, D], bf16, tag="xres")
                    nc.sync.dma_start(
                        out=xt[:], in_=xr_ext[t * P : (t + 1) * P, :]
                    )
                    nc.tensor.matmul(
                        out=pts[t][:],
                        lhsT=wd_sb[:, t, :],
                        rhs=xt[:],
                        start=not started[t],
                        stop=True,
                    )
                    ot = opool.tile([P, D], f32, tag="outp")
                    nc.scalar.activation(
                        out=ot[:], in_=pts[t][:],
                        func=mybir.ActivationFunctionType.Copy,
                        scale=inv_sb[:, t : t + 1],
                    )
                    nc.sync.dma_start(
                        out=out_ext[t * P : (t + 1) * P, :], in_=ot[:]
                    )

    nc.compile()
    return nc


def kernel(x, edge_index):
    from concourse.bass_utils import run_bass_kernel_spmd

    x = np.ascontiguousarray(np.asarray(x), dtype=np.float32)
    per_core, meta = preprocess(x, edge_index, N_CORES)
    nc = build_core_kernel(meta)
    res = run_bass_kernel_spmd(nc, per_core, core_ids=list(range(N_CORES)))

    N, D, TPC = meta["N"], meta["D"], meta["TPC"]
    out = np.empty((N, D), dtype=np.float32)
    pos_of_tile = meta["pos_of_tile"]
    core_of_tile = meta["core_of_tile"]
    for t in range(meta["T_total"]):
        c = core_of_tile[t]
        pos = pos_of_tile[t]
        r0 = t * P
        r1 = min(N, r0 + P)
        out[r0:r1] = res.results[c]["out"][pos * P : pos * P + (r1 - r0)]
    return out
